# revision 20
# baseline (speedup 1.0000x reference)
"""Trainium2 Bass kernel for GPT-Neo style causal attention.

reference:
    scores = q @ k.T              (no 1/sqrt(d) scaling), fp32
    scores = where(causal, scores, -inf)
    attn   = softmax(scores, -1)
    attn   = attn * ctx_mask[b, None, None, :]
    out    = attn @ v

Shapes: B=2, H=16, S=2048, D=128 fp32. Sharded over 8 cores by (b*h) —
4 heads per core; each core's heads belong to one batch, so one
ctx_mask row per core.

Per-core algorithm (T-layout softmax: k on partitions, q on free axis):
  - Q,K are loaded fp32 (parallel HWDGE rings), cast to fp16 into one
    combined [Q|K] tile (DVE), and transposed by a single whole-pair
    DMA XBAR transpose per head — the only user of the SWDGE scratch,
    so transposes never serialize against other DMAs.  No PE
    transposes, no fp32-family LDWEIGHTS anywhere.
  - per key-block t: scoresT[k,q] = KT_t.T @ QT (fp16, 1 cyc/col; FWL
    weight loads hide under the 512-col streams).
  - one exp() per strip on ScalarE with per-partition bias ln(ctx_mask):
    expT = exp(s - 16 + ln(cm_key)) = exp(s)*cm_key -> bf16.  Causal
    diagonal block masked post-exp by a 0/1 upper-tri multiply on DVE.
  - AV: out_psum[q, 0:129] = sum_kb expT_kb.T @ [V | 1/cm] (bf16,
    fp32 PSUM accum).  Column 128 accumulates exp*cm*(1/cm) = exp,
    i.e. the pre-ctx-mask softmax denominator -> reciprocal + scale.
  - cm clamped at 1e-30 so cm=0 stays exact.

Scheduling: each head's cast/transpose/vp prologue is emitted mid-way
through the previous head's loop (its loads are complete by then); the
last two AV blocks of each head are carried into the next head's first
two iterations to fill the exp-pipeline warmup bubble at head starts.
A dummy bf16 matmul burst warms the PE HAM clock gate to 2.4 GHz while
the first inputs load.
"""

from contextlib import ExitStack

import numpy as np

import concourse.bass as bass
import concourse.mybir as mybir
import concourse.tile as tile
from concourse.bass_utils import run_bass_kernel_spmd
from concourse.masks import make_upper_triangular

F32 = mybir.dt.float32
F16 = mybir.dt.float16
BF16 = mybir.dt.bfloat16

B, H, S, D = 2, 16, 2048, 128
NCORES = 8
NBH = (B * H) // NCORES  # heads per core


def _legalize_waits(nc):
    """This container's walrus accepts at most 1 sync wait per instruction
    (2 for EventSemaphore). Hoist extra waits onto same-engine NoOps
    inserted immediately before the offending instruction (semantically
    identical: all waits still complete before it executes)."""
    n = 0
    ctr = [0]
    for f in nc.m.functions:
        for bb in f.blocks:
            out = []
            dirty = False
            for inst in bb.instructions:
                si = inst.sync_info
                cap = 2 if isinstance(inst, mybir.InstEventSemaphore) else 1
                if si is not None and len(si.on_wait) > cap:
                    waits = list(si.on_wait)
                    extra, keep = waits[:-cap], waits[-cap:]
                    for w in extra:
                        ctr[0] += 1
                        nop = mybir.InstNoOp(
                            name=f"waitsplit-{ctr[0]}",
                            ins=[],
                            outs=[],
                            engine=inst.engine,
                            sync_info=mybir.SyncInfo(on_wait=[w], on_update=[]),
                        )
                        nc.register_instruction(nop, overwrite=True)
                        out.append(nop)
                    inst.sync_info = mybir.SyncInfo(
                        on_wait=keep, on_update=list(si.on_update)
                    )
                    dirty = True
                    n += 1
                out.append(inst)
            if dirty:
                bb.instructions = out
    return n


def build_nc(nbh=NBH, s=S, d=D, num_devices=NCORES):
    SB = s // 128  # 128-row blocks along the sequence
    nc = bass.Bass("TRN2", target_bir_lowering=False, debug=False,
                   num_devices=num_devices)
    q = nc.dram_tensor("q", [nbh, s, d], F32, kind="ExternalInput")
    k = nc.dram_tensor("k", [nbh, s, d], F32, kind="ExternalInput")
    v = nc.dram_tensor("v", [nbh, s, d], F32, kind="ExternalInput")
    cm = nc.dram_tensor("cm", [s], F32, kind="ExternalInput")
    o = nc.dram_tensor("out", [nbh, s, d], F32, kind="ExternalOutput")

    EXPFN = mybir.ActivationFunctionType.Exp
    LNFN = mybir.ActivationFunctionType.Ln

    with tile.TileContext(nc) as tc, ExitStack() as ctx:
        consts = ctx.enter_context(tc.tile_pool(name="consts", bufs=1))
        stage = ctx.enter_context(tc.tile_pool(name="stage", bufs=2))
        h16 = ctx.enter_context(tc.tile_pool(name="h16", bufs=2))
        tpool = ctx.enter_context(tc.tile_pool(name="tpool", bufs=2))
        vpool = ctx.enter_context(tc.tile_pool(name="vpool", bufs=2))
        # expT strips: the next head's t=0,1 exps overlap the carried AV
        # blocks of this head, so those two strips are double-buffered;
        # strips 2-15 are written only after the carried AVs retire.
        epoolE = ctx.enter_context(tc.tile_pool(name="epoolE", bufs=2))
        epoolL = ctx.enter_context(tc.tile_pool(name="epoolL", bufs=1))
        opool = ctx.enter_context(tc.tile_pool(name="opool", bufs=2))
        small = ctx.enter_context(tc.tile_pool(name="small", bufs=4))
        psum = ctx.enter_context(tc.tile_pool(name="psum", bufs=2, space="PSUM"))
        psav = ctx.enter_context(tc.tile_pool(name="psav", bufs=2, space="PSUM"))

        # 0/1 upper-triangular (incl diag) keep-mask for the causal
        # diagonal block, applied to expT (post-exp) in bf16.
        tri32 = consts.tile([128, 128], F32)
        make_upper_triangular(nc, tri32, val=1.0, diag=True)
        tri = consts.tile([128, 128], BF16)
        nc.vector.tensor_copy(tri, tri32)

        # ctx-mask pipeline: cmc = max(cm, 1e-30); lncm = ln(cmc) - 16
        # (exp bias); invc = 1/cmc in bf16 (denominator column of V')
        cmt = consts.tile([128, SB], F32)
        nc.sync.dma_start(out=cmt, in_=cm.ap().rearrange("(sb p) -> p sb", p=128))
        cmc = consts.tile([128, SB], F32)
        nc.vector.tensor_scalar_max(cmc, cmt, 1e-30)
        lncm = consts.tile([128, SB], F32)
        nc.scalar.activation(lncm, cmc, LNFN)
        nc.vector.tensor_scalar_add(lncm, lncm, -16.0)
        invc = consts.tile([128, SB], F32)
        nc.vector.reciprocal(invc, cmc)
        invcb = consts.tile([128, SB], BF16)
        nc.vector.tensor_copy(invcb, invc)

        # Dummy bf16 matmuls (values irrelevant) to warm the PE clock gate
        # while the first inputs load + cast + transpose.
        wpw = consts.tile([128, 128], BF16)
        nc.vector.memset(wpw, 1.0)
        wps = psav.tile([128, 256], F32, tag="av")
        for _ in range(200):
            nc.tensor.matmul(wps[:, 0:128], wpw, wpw, start=True, stop=True)

        qap, kap, vap, oap = q.ap(), k.ap(), v.ap(), o.ap()

        def loads(bh):
            qn = stage.tile([128, SB, d], F32, tag="qn")
            kn = stage.tile([128, SB, d], F32, tag="kn")
            vn = stage.tile([128, SB, d], F32, tag="vn")
            nc.sync.dma_start(out=qn, in_=qap[bh].rearrange("(sb p) d -> p sb d", p=128))
            nc.sync.dma_start(out=kn, in_=kap[bh].rearrange("(sb p) d -> p sb d", p=128))
            nc.sync.dma_start(out=vn, in_=vap[bh].rearrange("(sb p) d -> p sb d", p=128))
            return qn, kn, vn

        def mid(bh, qn, kn, vn):
            # fp16 casts into one combined [Q|K] tile, then a single
            # whole-pair XBAR transpose: qkt[d, 0, sb, qrow] = Q^T,
            # qkt[d, 1, sb, krow] = K^T.
            qk16 = h16.tile([128, 2, SB, d], F16, tag="qk16")
            nc.vector.tensor_copy(qk16[:, 0], qn)
            nc.vector.tensor_copy(qk16[:, 1], kn)
            qkt = tpool.tile([128, 2, SB, 128], F16, tag="qkt")
            nc.sync.dma_start_transpose(out=qkt, in_=qk16)
            # V' = [V | 1/cm] bf16
            vp = vpool.tile([128, SB, d + 1], BF16, tag="vp")
            nc.vector.tensor_copy(vp[:, :, 0:d], vn)
            nc.vector.tensor_copy(vp[:, :, d], invcb)
            return qkt, vp

        def tloop(bh, qkt, vp, carry, emit_next):
            expE = epoolE.tile([128, 2, s], BF16, tag="expE",
                               name=f"expE_{bh}")
            expL = epoolL.tile([128, SB - 2, s], BF16, tag="expL",
                               name=f"expL_{bh}")
            ostage = opool.tile([128, SB, d], F32, tag="ostage")

            def estrip(t):
                return expE[:, t] if t < 2 else expL[:, t - 2]

            def av_block(qb):
                av = psav.tile([128, 256], F32, tag="av")
                for kb in range(qb + 1):
                    nc.tensor.matmul(
                        av[:, 0:d + 1],
                        estrip(kb)[:, qb * 128:(qb + 1) * 128],
                        vp[:, kb, :],
                        start=(kb == 0),
                        stop=(kb == qb),
                    )
                rec = small.tile([128, 1], F32, tag="rec")
                nc.vector.reciprocal(rec, av[:, d:d + 1])
                nc.vector.tensor_scalar_mul(ostage[:, qb, :], av[:, 0:d], rec)

            # scores strips capped at 1536 cols (3 PSUM banks) so two strip
            # slots + the av pool fit in the 8 PSUM banks; the long
            # strips (t < 4) are split into two slots/exps.
            for t in range(SB):
                for (lo, hi) in (((t * 128) // 512 * 512,
                                  min(((t * 128) // 512 * 512) + 1536, s)),
                                 (min(((t * 128) // 512 * 512) + 1536, s), s)):
                    if lo >= hi:
                        continue
                    sc = psum.tile([128, 1536], F32, tag="ps")
                    qstart = max(t * 128, lo)
                    while qstart < hi:
                        seg = min(512 - (qstart % 512), hi - qstart)
                        b0, b1 = qstart // 128, (qstart + seg) // 128
                        nc.tensor.matmul(
                            sc[:, qstart - lo:qstart - lo + seg],
                            qkt[:, 1, t, :],
                            qkt[:, 0, b0:b1, :],
                            start=True,
                            stop=True,
                        )
                        qstart += seg
                    q0 = max(t * 128, lo)
                    # exp(s - 16 + ln(cm_key)) -> bf16
                    nc.scalar.activation(estrip(t)[:, q0:hi], sc[:, q0 - lo:hi - lo],
                                         EXPFN, bias=lncm[:, t:t + 1])
                # causal-mask the diagonal block post-exp (0/1 multiply);
                # only the last (kb==qb) AV pair of av_block(t) waits on it
                nc.vector.tensor_mul(estrip(t)[:, t * 128:(t + 1) * 128],
                                     estrip(t)[:, t * 128:(t + 1) * 128], tri)
                # the previous head's last two AV blocks fill the PE while
                # this head's first exp strips drain
                if t <= 1 and carry is not None:
                    carry[t]()
                if t >= 2:
                    av_block(t - 2)
                if t == 5 and emit_next is not None:
                    # next head's cast/transpose/vp prologue, emitted
                    # mid-loop: its input loads have completed by now, so
                    # it slots into engine queues without blocking them
                    emit_next()

            # chunked stores for the blocks normalized inside this loop;
            # blocks 14,15 are stored by the carry closures below (their
            # normalizes happen during the next head's loop, and a store
            # emitted before its writer would not be ordered after it)
            for g0, gs in ((0, 4), (4, 4), (8, 4), (12, 2)):
                nc.sync.dma_start(
                    out=oap[bh][g0 * 128:(g0 + gs) * 128].rearrange(
                        "(sb p) d -> p sb d", p=128),
                    in_=ostage[:, g0:g0 + gs, :],
                )

            def carry_av(qb):
                av_block(qb)
                nc.sync.dma_start(
                    out=oap[bh][qb * 128:(qb + 1) * 128].rearrange(
                        "(sb p) d -> p sb d", p=128),
                    in_=ostage[:, qb:qb + 1, :],
                )
            return [lambda: carry_av(SB - 2), lambda: carry_av(SB - 1)]

        hnd = {0: loads(0)}
        mids = {0: mid(0, *hnd[0])}
        carry = None
        for bh in range(nbh):
            if bh + 1 < nbh:
                hnd[bh + 1] = loads(bh + 1)

                def emit_next(b=bh + 1):
                    mids[b] = mid(b, *hnd[b])
            else:
                emit_next = None
            carry = tloop(bh, *mids[bh], carry, emit_next)
        carry[0]()
        carry[1]()

    _legalize_waits(nc)
    return nc


_nc_cache = {}


def _get_nc():
    key = (NBH, S, D)
    if key not in _nc_cache:
        _nc_cache[key] = build_nc()
    return _nc_cache[key]


def kernel(query, key, value, ctx_mask):
    q = np.ascontiguousarray(query, dtype=np.float32).reshape(B * H, S, D)
    k = np.ascontiguousarray(key, dtype=np.float32).reshape(B * H, S, D)
    v = np.ascontiguousarray(value, dtype=np.float32).reshape(B * H, S, D)
    cmf = np.ascontiguousarray(ctx_mask, dtype=np.float32)

    in_maps = []
    for c in range(NCORES):
        lo = c * NBH
        in_maps.append({
            "q": q[lo:lo + NBH],
            "k": k[lo:lo + NBH],
            "v": v[lo:lo + NBH],
            "cm": cmf[(lo // H)],
        })
    nc = _get_nc()
    res = run_bass_kernel_spmd(nc, in_maps, list(range(NCORES)))
    outs = [res.results[c]["out"] for c in range(NCORES)]
    return np.concatenate(outs, axis=0).reshape(B, H, S, D).astype(np.float32)
